# revision 48
# baseline (speedup 1.0000x reference)
"""Distributed MultiHeadAttention kernel for 8 Trainium2 NeuronCores.

Problem: B=2, L=2048, D=1024, H=16 heads (DH=64), causal attn_mask +
key_padding_mask, torch-Linear-convention projections.

Sharding: core = (batch b = core//4, group rank j = core%4). Each core
projects q/k/v for its batch restricted to its 4 heads (256 channels),
runs streaming softmax attention in a [key, query]-transposed layout
(no max subtraction -- scores are O(1); masked scores get -1e5 added so
exp underflows to exactly 0), normalizes the attention tensor locally
with the row-sum obtained from an appended ones-column in the V matmul,
AllGathers the normalized tensor within each 4-core group (two
pipelined collectives, one per head pair, triggered incrementally as
the last query chunk's normalization lands), and computes the output
projection for its own 512 rows. Host assembles [2, 2048, 1024].

The kernel is close to DMA-bandwidth bound (~27 MB of HBM traffic over
3 engine DMA queues), so inputs are bf16, masks/outputs bf16, and x is
loaded as full-row 4 KB-line transfers with per-contraction-block
dependency granularity. The attention phase is an ACT(exp)-bound
ping-pong; leaving the PE idle in its gaps makes the HAM clock-gate
throttle every matmul to half clock, so projection work is *woven into*
the attention instruction stream: K is projected up front, Q/V chunks
for query-chunk qc+1 are emitted between the score/AV matmuls of chunk
qc (head pair 0), and the pair-0 half of the output projection runs
inside pair 1's attention, staged to SBUF and completed after pair 1's
AllGather lands.
"""
import os
import sys

sys.path.insert(0, '/opt/trn_rl_repo')

import numpy as np
import ml_dtypes

import concourse.bass as bass
import concourse.bacc as bacc
import concourse.mybir as mybir
import concourse.tile as tile
from concourse.bass_utils import run_bass_kernel_spmd

BF16 = mybir.dt.bfloat16
F32 = mybir.dt.float32
NPBF16 = ml_dtypes.bfloat16

B, L, D, H = 2, 2048, 1024, 16
DH = D // H                      # 64
N_CORES = 8
GROUPS = [[0, 1, 2, 3], [4, 5, 6, 7]]
HPC = H // 4                     # heads per core = 4
CPC = HPC * DH                   # channels per core = 256
LPC = L // 4                     # output rows per core = 512
QC = 512                         # query-chunk size
NQC = L // QC                    # 4
KB = 128                         # key-block size
NKB = L // KB                    # 16
NDB = D // 128                   # 8 contraction blocks
MASK_VAL = -1e5                  # exp(MASK_VAL/8 + s) == 0 in fp32

ExpFn = mybir.ActivationFunctionType.Exp

_PROG_CACHE = {}
last_results = None


def _analyze_masks(attn_mask, key_padding_mask):
    """Derive the per-query-chunk (kb, q0, mask) records plus per-batch
    additive mask tiles from the actual boolean mask inputs."""
    am = np.asarray(attn_mask, dtype=bool)
    kpm = np.asarray(key_padding_mask, dtype=bool)
    cm = [am | kpm[b][None, :] for b in range(B)]     # [L, L], True = masked

    for b in range(B):
        if cm[b].all(axis=1).any():
            return None, None, True

    structure = []
    mask_chunks = [[] for _ in range(B)]
    off = 0
    for qc in range(NQC):
        recs = []
        for kb in range(NKB):
            subs = [cm[b][qc * QC:(qc + 1) * QC, kb * KB:(kb + 1) * KB]
                    for b in range(B)]                 # [QC, 128]
            allowed = [~s.all(axis=1) for s in subs]
            union = allowed[0] | allowed[1]
            if not union.any():
                continue
            q0 = int(np.argmax(union))
            if not union[q0:].all():
                q0 = 0
            mask_cols = [s[q0:].any(axis=1) for s in subs]
            any_mask = any(mc.any() for mc in mask_cols)
            mask_rec = None
            if any_mask:
                firsts = [int(np.argmax(mc)) for mc in mask_cols if mc.any()]
                lasts = [QC - q0 - int(np.argmax(mc[::-1])) for mc in mask_cols
                         if mc.any()]
                c0 = q0 + min(firsts)
                c1 = q0 + max(lasts)
                w = c1 - c0
                for b in range(B):
                    sub = subs[b][c0:c1, :]
                    tileM = np.where(sub.T, np.float32(MASK_VAL),
                                     np.float32(0.0))  # [128, w]
                    mask_chunks[b].append(tileM)
                mask_rec = (off, c0, w)
                off += w
            recs.append((kb, q0, mask_rec))
        if not recs:
            return None, None, True
        # PSUM start-flag coverage: the first record must cover the full
        # chunk and q0 must be non-decreasing across kb.
        if recs[0][1] != 0:
            return None, None, True
        q0s = [q0 for _, q0, _ in recs]
        if any(a > b2 for a, b2 in zip(q0s, q0s[1:])):
            return None, None, True
        structure.append(recs)

    mw = max(off, 1)
    mask_bufs = []
    for b in range(B):
        buf = np.zeros((128, mw), dtype=np.float32)
        o = 0
        for tileM in mask_chunks[b]:
            buf[:, o:o + tileM.shape[1]] = tileM
            o += tileM.shape[1]
        mask_bufs.append(buf)
    return structure, mask_bufs, False


def _structure_key(structure, mw):
    return (mw, tuple(tuple((kb, q0, mask) for kb, q0, mask in recs)
                      for recs in structure))


def _build_program(structure, mw):
    """Build the SPMD Bass program (identical on all 8 cores)."""
    nc = bacc.Bacc("TRN2", target_bir_lowering=False, debug=False,
                   num_devices=N_CORES)

    xqT = nc.declare_dram_parameter("xqT", [D, L], BF16, isOutput=False)
    xkT = nc.declare_dram_parameter("xkT", [D, L], BF16, isOutput=False)
    xvT = nc.declare_dram_parameter("xvT", [D, L], BF16, isOutput=False)
    wqT = nc.declare_dram_parameter("wqT", [D, CPC], BF16, isOutput=False)
    wkT = nc.declare_dram_parameter("wkT", [D, CPC], BF16, isOutput=False)
    wvT = nc.declare_dram_parameter("wvT", [D, CPC], BF16, isOutput=False)
    woT = nc.declare_dram_parameter("woT", [D, D], BF16, isOutput=False)
    bq_in = nc.declare_dram_parameter("bq", [128, 2], F32, isOutput=False)
    bk_in = nc.declare_dram_parameter("bk", [128, 2], F32, isOutput=False)
    bv_in = nc.declare_dram_parameter("bv", [1, CPC], BF16, isOutput=False)
    bo_in = nc.declare_dram_parameter("bo", [1, D], BF16, isOutput=False)
    masks_in = nc.declare_dram_parameter("masks", [128, mw], BF16, isOutput=False)
    out = nc.declare_dram_parameter("out", [LPC, D], BF16, isOutput=True)

    # per head-pair AllGather bounce buffers (normalized attn)
    ag_in = [nc.dram_tensor(f"ag_in{p}", [128, L], BF16) for p in range(2)]
    ag_out = [nc.dram_tensor(f"ag_out{p}", [4, 128, L], BF16)
              for p in range(2)]
    r_dram = [nc.dram_tensor(f"r_dram{p}", [8, QC], BF16) for p in range(2)]

    with tile.TileContext(nc, num_cores=N_CORES) as tc:
        with tc.tile_pool(name="persist", bufs=1) as pers:
            wq_sb = pers.tile([128, NDB, CPC], BF16, tag="wq")
            wk_sb = pers.tile([128, NDB, CPC], BF16, tag="wk")
            wv_sb = pers.tile([128, NDB, CPC], BF16, tag="wv")
            wo_sb = pers.tile([128, NDB, D], BF16, tag="wo")
            bq_sb = pers.tile([128, 2], F32, tag="bq")
            bk_sb = pers.tile([128, 2], F32, tag="bk")
            bv_sb = pers.tile([1, CPC], BF16, tag="bv")
            bo_sb = pers.tile([1, D], BF16, tag="bo")
            masks_sb = pers.tile([128, mw], BF16, tag="masks")
            ones_sb = pers.tile([1, 128], BF16, tag="ones")
            qT_sb = pers.tile([128, 2, L], BF16, tag="qT")
            kT_sb = pers.tile([128, 2, L], BF16, tag="kT")
            v_sb = pers.tile([128, NKB, HPC, DH + 1], BF16, tag="v")
            ob0_sb = pers.tile([128, 4, 2, 512], BF16, tag="ob0")

            nc.sync.dma_start(
                out=wk_sb[:], in_=wkT.ap().rearrange("(db p) c -> p db c", p=128))
            nc.sync.dma_start(
                out=wq_sb[:], in_=wqT.ap().rearrange("(db p) c -> p db c", p=128))
            nc.gpsimd.dma_start(
                out=wv_sb[:], in_=wvT.ap().rearrange("(db p) c -> p db c", p=128))
            nc.sync.dma_start(out=bq_sb[:], in_=bq_in[:])
            nc.sync.dma_start(out=bk_sb[:], in_=bk_in[:])
            nc.sync.dma_start(out=bv_sb[:], in_=bv_in[:])
            nc.sync.dma_start(out=bo_sb[:], in_=bo_in[:])
            nc.vector.memset(ones_sb[:], 1.0)
            nc.vector.memset(v_sb[:, :, :, DH:DH + 1], 1.0)
            # per-engine register copies of (rank % 4) * 512 for the
            # phase-O AllGather slice loads
            l0r_e = {}
            for _eng in (nc.sync, nc.scalar):
                l0r_e[_eng.engine] = (_eng.partition_id() % 4) * 512

            # masks early on sync (small, needed at attention start); wo
            # is emitted after the x chunk loads so it doesn't delay them
            nc.sync.dma_start(out=masks_sb[:], in_=masks_in[:])

            # x inputs, loaded as one full-L DMA per 128-channel
            # contraction block: 4 KB contiguous lines (fast) and
            # per-db dependency granularity (first matmuls start early)
            with tc.tile_pool(name="xt", bufs=1) as xtp:
                xq_f = xtp.tile([128, NDB, L], BF16, tag="xq")
                xk_f = xtp.tile([128, NDB, L], BF16, tag="xk")
                xv_f = xtp.tile([128, NDB, L], BF16, tag="xv")
                for db in range(NDB):
                    nc.scalar.dma_start(
                        out=xk_f[:, db, :],
                        in_=xkT.ap().rearrange("(db p) l -> p db l",
                                               p=128)[:, db, :])
                for db in range(NDB):
                    nc.sync.dma_start(
                        out=xq_f[:, db, :],
                        in_=xqT.ap().rearrange("(db p) l -> p db l",
                                               p=128)[:, db, :])
                    nc.gpsimd.dma_start(
                        out=xv_f[:, db, :],
                        in_=xvT.ap().rearrange("(db p) l -> p db l",
                                               p=128)[:, db, :])
                wo_r = woT.ap().rearrange("(db p) c -> p db c", p=128)
                nc.scalar.dma_start(out=wo_sb[:, 0:4, :], in_=wo_r[:, 0:4, :])
                nc.gpsimd.dma_start(out=wo_sb[:, 4:8, :], in_=wo_r[:, 4:8, :])

                # ---- emission helpers ----
                def emit_qk_cb(psp, w_sb, b_sb, t_sb, xs, cb, lcs, nm):
                    """One output-channel block (128 ch) of a q/k
                    projection for the given l-chunks; weight stationary."""
                    pss = [psp.tile([128, QC], F32, tag="psqk",
                                    name=f"psqk_{nm}_{cb}_{lc}")
                           for lc in lcs]
                    for db in range(NDB):
                        for i, lc in enumerate(lcs):
                            nc.tensor.matmul(
                                pss[i][:],
                                lhsT=w_sb[:, db, cb * 128:(cb + 1) * 128],
                                rhs=xs[:, db, lc * QC:(lc + 1) * QC],
                                start=(db == 0), stop=(db == NDB - 1))
                    for i, lc in enumerate(lcs):
                        nc.vector.tensor_scalar_add(
                            t_sb[:, cb, lc * QC:(lc + 1) * QC], pss[i][:],
                            b_sb[:, cb:cb + 1])

                def emit_v_ls(psp, lc, ls):
                    """One 128-key block of the V projection."""
                    kbg = lc * 4 + ls
                    psv = psp.tile([128, CPC], F32, tag="psv",
                                   name=f"psv_{kbg}")
                    for db in range(NDB):
                        nc.tensor.matmul(
                            psv[:],
                            lhsT=xv_f[:, db, kbg * 128:(kbg + 1) * 128],
                            rhs=wv_sb[:, db, :],
                            start=(db == 0), stop=False)
                    nc.tensor.matmul(
                        psv[:], lhsT=ones_sb[:, 0:128], rhs=bv_sb[:],
                        start=False, stop=True)
                    nc.vector.tensor_copy(
                        v_sb[:, kbg, :, 0:DH],
                        psv[:].rearrange("p (h d) -> p h d", h=HPC))

                # ---- upfront: K (all), Q(0), V(0) ----
                ctxP = nc.named_scope("phaseP"); ctxP.__enter__()
                with tc.tile_pool(name="psP", bufs=5, space="PSUM") as psP, \
                     tc.tile_pool(name="psPV", bufs=2, space="PSUM") as psPV:
                    for cb in range(2):
                        emit_qk_cb(psP, wk_sb, bk_sb, kT_sb, xk_f, cb,
                                   [0, 1, 2, 3], "k")
                    for cb in range(2):
                        emit_qk_cb(psP, wq_sb, bq_sb, qT_sb, xq_f, cb, [0],
                                   "q0")
                    for ls in range(4):
                        emit_v_ls(psPV, 0, ls)
                ctxP.__exit__(None, None, None)

                # ---- attention with woven projection / O-proj work ----
                ctxA = nc.named_scope("phaseA"); ctxA.__enter__()
                with tc.tile_pool(name="ex", bufs=4) as exp_pool, \
                     tc.tile_pool(name="araw", bufs=2) as arawp, \
                     tc.tile_pool(name="sm", bufs=2) as smalls, \
                     tc.tile_pool(name="psS", bufs=2, space="PSUM") as psS, \
                     tc.tile_pool(name="psA", bufs=2, space="PSUM") as psA, \
                     tc.tile_pool(name="psF", bufs=1, space="PSUM") as psF, \
                     tc.tile_pool(name="psF2", bufs=1, space="PSUM") as psF2:

                    fat0 = arawp.tile([128, 4, QC], BF16, tag="fat0")

                    def fill_p0(qc):
                        """Filler chunks during pair-0 attention of chunk
                        qc: project Q/V for chunk qc+1."""
                        if qc >= 3:
                            return []
                        chunks = []
                        for cb in range(2):
                            chunks.append(lambda cb=cb: emit_qk_cb(
                                psF, wq_sb, bq_sb, qT_sb, xq_f, cb,
                                [qc + 1], f"q{qc + 1}"))
                        for ls in range(4):
                            chunks.append(lambda ls=ls: emit_v_ls(
                                psF2, qc + 1, ls))
                        return chunks

                    def o_stage0(ls, nch):
                        """Pair-0 half of the output projection for one
                        output tile; staged to SBUF. Reuses the psqk ring
                        (same shape, disjoint phase) to stay at 8 banks."""
                        po = psF.tile([128, 512], F32, tag="psqk",
                                      name=f"po0_{ls}_{nch}")
                        for r in range(4):
                            nc.tensor.matmul(
                                po[:],
                                lhsT=fat0[:, r, ls * 128:(ls + 1) * 128],
                                rhs=wo_sb[:, 2 * r, nch * 512:(nch + 1) * 512],
                                start=(r == 0), stop=(r == 3))
                        nc.vector.tensor_copy(ob0_sb[:, ls, nch, :], po[:])

                    def fill_p1(qc):
                        if qc == 0:
                            return []
                        # distribute the 8 (ls, nch) tiles: 3 on qc=1,
                        # 3 on qc=2, 2 on qc=3
                        alloc = {1: [(0, 0), (0, 1), (1, 0)],
                                 2: [(1, 1), (2, 0), (2, 1)],
                                 3: [(3, 0), (3, 1)]}[qc]
                        return [lambda ls=ls, nch=nch: o_stage0(ls, nch)
                                for (ls, nch) in alloc]

                    for p in range(2):
                        if p == 1:
                            # own query-slice of pair-0 attn from each rank
                            l0e = l0r_e[nc.sync.engine]
                            for r in range(4):
                                nc.sync.dma_start(
                                    out=fat0[:, r, :],
                                    in_=ag_out[0][r, :, bass.ds(l0e, QC)])
                        for qc in range(NQC):
                            recs = structure[qc]
                            first_kb = recs[0][0]
                            last_kb = recs[-1][0]
                            fillers = fill_p0(qc) if p == 0 else fill_p1(qc)
                            fi = 0
                            n_recs = len(recs)
                            pa = {hp: psA.tile([65, QC], F32, tag="pa",
                                               name=f"pa_{p}_{qc}_{hp}")
                                  for hp in range(2)}
                            for ri, (kb, q0, mask) in enumerate(recs):
                                ps2 = psS.tile([128, 2, QC], F32, tag="ps2",
                                               name=f"ps2_{p}_{qc}_{kb}")
                                for hp in range(2):
                                    h = p * 2 + hp
                                    hb, hoff = h // 2, (h % 2) * 64
                                    nc.tensor.matmul(
                                        ps2[:, hp, q0:],
                                        lhsT=kT_sb[hoff:hoff + 64, hb,
                                                   kb * KB:(kb + 1) * KB],
                                        rhs=qT_sb[hoff:hoff + 64, hb,
                                                  qc * QC + q0:(qc + 1) * QC],
                                        start=True, stop=True)
                                if mask is not None:
                                    off, c0, wm = mask
                                    for hp in range(2):
                                        nc.vector.tensor_add(
                                            ps2[:, hp, c0:c0 + wm],
                                            ps2[:, hp, c0:c0 + wm],
                                            masks_sb[:, off:off + wm])
                                ex2 = exp_pool.tile([128, 2, QC], BF16,
                                                    tag="ex",
                                                    name=f"ex_{p}_{qc}_{kb}")
                                nc.scalar.activation(
                                    out=ex2[:, :, q0:], in_=ps2[:, :, q0:],
                                    func=ExpFn, scale=0.125)
                                # independent PE filler lands in the
                                # exp-wait gap before this rec's AV
                                while (fi < len(fillers)
                                       and fi <= (ri * len(fillers))
                                       // n_recs):
                                    fillers[fi](); fi += 1
                                for hp in range(2):
                                    h = p * 2 + hp
                                    nc.tensor.matmul(
                                        pa[hp][:, q0:],
                                        lhsT=v_sb[:, kb, h, :],
                                        rhs=ex2[:, hp, q0:],
                                        start=(kb == first_kb),
                                        stop=(kb == last_kb))
                            while fi < len(fillers):
                                fillers[fi](); fi += 1
                            # incremental normalize + ship for this qc so
                            # the AllGather can trigger right after the
                            # last attention matmul of the pair
                            araw2 = arawp.tile([64, 2, QC], BF16,
                                               tag="araw",
                                               name=f"araw_{p}_{qc}")
                            for hp in range(2):
                                nc.vector.tensor_copy(
                                    araw2[:, hp, :], pa[hp][0:64, :])
                                # 1/S straight from the PSUM row; the DMA
                                # moves it to the r_dram slot (engines
                                # cannot shift partitions; DMA can)
                                rt = smalls.tile(
                                    [65, QC], BF16, tag="rt",
                                    name=f"rt_{p}_{qc}_{hp}")
                                with nc.allow_low_precision(
                                        reason="1/S to bf16; 0.4% on "
                                               "softmax norm is within "
                                               "tolerance"):
                                    nc.vector.reciprocal(
                                        rt[64:65, :], pa[hp][64:65, :])
                                nc.sync.dma_start(
                                    out=r_dram[p][qc * 2 + hp:
                                                  qc * 2 + hp + 1, :],
                                    in_=rt[64:65, :])
                            bc2 = arawp.tile([64, 2, QC], BF16, tag="bc",
                                             name=f"bc_{p}_{qc}")
                            nc.sync.dma_start(
                                out=bc2[:],
                                in_=bass.AP(tensor=r_dram[p],
                                            offset=qc * 2 * QC,
                                            ap=[[0, 64], [QC, 2], [1, QC]]))
                            fn2 = arawp.tile([64, 2, QC], BF16, tag="fn",
                                             name=f"fn_{p}_{qc}")
                            nc.vector.tensor_mul(
                                fn2[:].rearrange("p a l -> p (a l)"),
                                araw2[:].rearrange("p a l -> p (a l)"),
                                bc2[:].rearrange("p a l -> p (a l)"))
                            for hp in range(2):
                                nc.gpsimd.dma_start(
                                    out=ag_in[p][hp * 64:(hp + 1) * 64,
                                                 qc * QC:(qc + 1) * QC],
                                    in_=fn2[:, hp, :])
                        nc.gpsimd.collective_compute(
                            "AllGather", mybir.AluOpType.bypass,
                            replica_groups=GROUPS,
                            ins=[ag_in[p][:]], outs=[ag_out[p][:]])

                ctxA.__exit__(None, None, None)
            # ---------------- Phase O: finish output projection ---------
            ctxO = nc.named_scope("phaseO"); ctxO.__enter__()
            with tc.tile_pool(name="fat", bufs=1) as fatp, \
                 tc.tile_pool(name="ob", bufs=4) as obp, \
                 tc.tile_pool(name="psO", bufs=4, space="PSUM") as psO:
                fat1 = fatp.tile([128, 4, QC], BF16, tag="fat1")
                l0e = l0r_e[nc.scalar.engine]
                for r in range(4):
                    nc.scalar.dma_start(
                        out=fat1[:, r, :],
                        in_=ag_out[1][r, :, bass.ds(l0e, QC)])
                for ls in range(4):
                    for nch in range(2):
                        po = psO.tile([128, 512], F32, tag="po1",
                                      name=f"po1_{ls}_{nch}")
                        for r in range(4):
                            nc.tensor.matmul(
                                po[:],
                                lhsT=fat1[:, r, ls * 128:(ls + 1) * 128],
                                rhs=wo_sb[:, 2 * r + 1,
                                          nch * 512:(nch + 1) * 512],
                                start=(r == 0), stop=False)
                        nc.tensor.matmul(
                            po[:], lhsT=ones_sb[:, 0:128],
                            rhs=bo_sb[:, nch * 512:(nch + 1) * 512],
                            start=False, stop=True)
                        ob = obp.tile([128, 512], BF16, tag="ob",
                                      name=f"ob_{ls}_{nch}")
                        nc.vector.tensor_add(
                            ob[:], po[:], ob0_sb[:, ls, nch, :])
                        oeng = nc.sync if nch == 0 else nc.scalar
                        oeng.dma_start(
                            out=out[ls * 128:(ls + 1) * 128,
                                    nch * 512:(nch + 1) * 512],
                            in_=ob[:])

    ctxO.__exit__(None, None, None)
    nc.compile()
    return nc


def _host_fallback(query, key, value, attn_mask, key_padding_mask,
                   Wq, bq, Wk, bk, Wv, bv, Wo, bo):
    """Exact fp32 numpy replica of the reference (degenerate masks only)."""
    q = (query @ Wq.T + bq).reshape(B, L, H, DH).transpose(0, 2, 1, 3)
    k = (key @ Wk.T + bk).reshape(B, L, H, DH).transpose(0, 2, 1, 3)
    v = (value @ Wv.T + bv).reshape(B, L, H, DH).transpose(0, 2, 1, 3)
    scores = np.einsum('bhqd,bhkd->bhqk', q, k) / np.sqrt(np.float32(DH))
    scores = np.where(key_padding_mask[:, None, None, :], -1e30, scores)
    scores = np.where(attn_mask[None, None, :, :], -1e30, scores)
    scores = scores - scores.max(axis=-1, keepdims=True)
    w = np.exp(scores)
    w = w / w.sum(axis=-1, keepdims=True)
    attn = np.einsum('bhqk,bhkd->bhqd', w, v)
    attn = attn.transpose(0, 2, 1, 3).reshape(B, L, D)
    return (attn @ Wo.T + bo).astype(np.float32)


def kernel(query, key, value, attn_mask, key_padding_mask,
           Wq, bq, Wk, bk, Wv, bv, Wo, bo):
    global last_results
    query = np.asarray(query, dtype=np.float32)
    key = np.asarray(key, dtype=np.float32)
    value = np.asarray(value, dtype=np.float32)
    attn_mask = np.asarray(attn_mask, dtype=bool)
    key_padding_mask = np.asarray(key_padding_mask, dtype=bool)
    Wq, bq = np.asarray(Wq, np.float32), np.asarray(bq, np.float32)
    Wk, bk = np.asarray(Wk, np.float32), np.asarray(bk, np.float32)
    Wv, bv = np.asarray(Wv, np.float32), np.asarray(bv, np.float32)
    Wo, bo = np.asarray(Wo, np.float32), np.asarray(bo, np.float32)

    structure, mask_bufs, degenerate = _analyze_masks(attn_mask,
                                                      key_padding_mask)
    if degenerate:
        return _host_fallback(query, key, value, attn_mask, key_padding_mask,
                              Wq, bq, Wk, bk, Wv, bv, Wo, bo)

    mw = mask_bufs[0].shape[1]
    key_sig = _structure_key(structure, mw)
    if key_sig not in _PROG_CACHE:
        _PROG_CACHE[key_sig] = _build_program(structure, mw)
    nc = _PROG_CACHE[key_sig]

    woT_np = np.ascontiguousarray(Wo.T).astype(NPBF16)
    bo_np = bo.reshape(1, D).astype(NPBF16)
    xT_bf = [np.ascontiguousarray(a.transpose(0, 2, 1)).astype(NPBF16)
             for a in (query, key, value)]             # [B, D, L] bf16

    in_maps = []
    for core in range(N_CORES):
        b, j = divmod(core, 4)
        csl = slice(j * CPC, (j + 1) * CPC)
        in_maps.append({
            "xqT": xT_bf[0][b],
            "xkT": xT_bf[1][b],
            "xvT": xT_bf[2][b],
            "wqT": np.ascontiguousarray(Wq[csl, :].T).astype(NPBF16),
            "wkT": np.ascontiguousarray(Wk[csl, :].T).astype(NPBF16),
            "wvT": np.ascontiguousarray(Wv[csl, :].T).astype(NPBF16),
            "woT": woT_np,
            "bq": np.ascontiguousarray(bq[csl].reshape(2, 128).T),
            "bk": np.ascontiguousarray(bk[csl].reshape(2, 128).T),
            "bv": bv[csl].reshape(1, CPC).astype(NPBF16),
            "bo": bo_np,
            "masks": mask_bufs[b].astype(NPBF16),
        })

    trace = os.environ.get("KERNEL_TRACE", "0") == "1"
    res = run_bass_kernel_spmd(nc, in_maps, list(range(N_CORES)), trace=trace)
    last_results = res

    out = np.empty((B, L, D), dtype=np.float32)
    for core in range(N_CORES):
        b, j = divmod(core, 4)
        out[b, j * LPC:(j + 1) * LPC, :] = res.results[core]["out"].astype(np.float32)
    return out


# revision 52
# speedup vs baseline: 1.0683x; 1.0683x over previous
"""Distributed MultiHeadAttention kernel for 8 Trainium2 NeuronCores.

Problem: B=2, L=2048, D=1024, H=16 heads (DH=64), causal attn_mask +
key_padding_mask, torch-Linear-convention projections.

Sharding: core = (batch b = core//4, group rank j = core%4). Each core
projects q/k/v for its batch restricted to its 4 heads (256 channels),
runs streaming softmax attention in a [key, query]-transposed layout
(no max subtraction -- scores are O(1); masked scores get -1e5 added so
exp underflows to exactly 0), normalizes the attention tensor locally
with the row-sum obtained from an appended ones-column in the V matmul,
AllGathers the normalized tensor within each 4-core group (two
pipelined collectives, one per head pair, triggered incrementally as
the last query chunk's normalization lands), and computes the output
projection for its own 512 rows. Host assembles [2, 2048, 1024].

The kernel is close to DMA-bandwidth bound (~27 MB of HBM traffic over
3 engine DMA queues), so inputs are bf16, masks/outputs bf16, and x is
loaded as full-row 4 KB-line transfers with per-contraction-block
dependency granularity. The attention phase is an ACT(exp)-bound
ping-pong; leaving the PE idle in its gaps makes the HAM clock-gate
throttle every matmul to half clock, so projection work is *woven into*
the attention instruction stream: K is projected up front, Q/V chunks
for query-chunk qc+1 are emitted between the score/AV matmuls of chunk
qc (head pair 0), and the pair-0 half of the output projection runs
inside pair 1's attention, staged to SBUF and completed after pair 1's
AllGather lands.
"""
import os
import sys

sys.path.insert(0, '/opt/trn_rl_repo')

import numpy as np
import ml_dtypes

import concourse.bass as bass
import concourse.bacc as bacc
import concourse.mybir as mybir
import concourse.tile as tile
from concourse.bass_utils import run_bass_kernel_spmd

BF16 = mybir.dt.bfloat16
F32 = mybir.dt.float32
NPBF16 = ml_dtypes.bfloat16

B, L, D, H = 2, 2048, 1024, 16
DH = D // H                      # 64
N_CORES = 8
GROUPS = [[0, 1, 2, 3], [4, 5, 6, 7]]
HPC = H // 4                     # heads per core = 4
CPC = HPC * DH                   # channels per core = 256
LPC = L // 4                     # output rows per core = 512
QC = 512                         # query-chunk size
NQC = L // QC                    # 4
KB = 128                         # key-block size
NKB = L // KB                    # 16
NDB = D // 128                   # 8 contraction blocks
MASK_VAL = -1e5                  # exp(MASK_VAL/8 + s) == 0 in fp32

ExpFn = mybir.ActivationFunctionType.Exp

_PROG_CACHE = {}
last_results = None


def _analyze_masks(attn_mask, key_padding_mask):
    """Derive the per-query-chunk (kb, q0, mask) records plus per-batch
    additive mask tiles from the actual boolean mask inputs."""
    am = np.asarray(attn_mask, dtype=bool)
    kpm = np.asarray(key_padding_mask, dtype=bool)
    cm = [am | kpm[b][None, :] for b in range(B)]     # [L, L], True = masked

    for b in range(B):
        if cm[b].all(axis=1).any():
            return None, None, True

    structure = []
    mask_chunks = [[] for _ in range(B)]
    off = 0
    for qc in range(NQC):
        recs = []
        for kb in range(NKB):
            subs = [cm[b][qc * QC:(qc + 1) * QC, kb * KB:(kb + 1) * KB]
                    for b in range(B)]                 # [QC, 128]
            allowed = [~s.all(axis=1) for s in subs]
            union = allowed[0] | allowed[1]
            if not union.any():
                continue
            q0 = int(np.argmax(union))
            if not union[q0:].all():
                q0 = 0
            mask_cols = [s[q0:].any(axis=1) for s in subs]
            any_mask = any(mc.any() for mc in mask_cols)
            mask_rec = None
            if any_mask:
                firsts = [int(np.argmax(mc)) for mc in mask_cols if mc.any()]
                lasts = [QC - q0 - int(np.argmax(mc[::-1])) for mc in mask_cols
                         if mc.any()]
                c0 = q0 + min(firsts)
                c1 = q0 + max(lasts)
                w = c1 - c0
                for b in range(B):
                    sub = subs[b][c0:c1, :]
                    tileM = np.where(sub.T, np.float32(MASK_VAL),
                                     np.float32(0.0))  # [128, w]
                    mask_chunks[b].append(tileM)
                mask_rec = (off, c0, w)
                off += w
            recs.append((kb, q0, mask_rec))
        if not recs:
            return None, None, True
        # PSUM start-flag coverage: the first record must cover the full
        # chunk and q0 must be non-decreasing across kb.
        if recs[0][1] != 0:
            return None, None, True
        q0s = [q0 for _, q0, _ in recs]
        if any(a > b2 for a, b2 in zip(q0s, q0s[1:])):
            return None, None, True
        structure.append(recs)

    mw = max(off, 1)
    mask_bufs = []
    for b in range(B):
        buf = np.zeros((128, mw), dtype=np.float32)
        o = 0
        for tileM in mask_chunks[b]:
            buf[:, o:o + tileM.shape[1]] = tileM
            o += tileM.shape[1]
        mask_bufs.append(buf)
    return structure, mask_bufs, False


def _structure_key(structure, mw):
    return (mw, tuple(tuple((kb, q0, mask) for kb, q0, mask in recs)
                      for recs in structure))


def _build_program(structure, mw):
    """Build the SPMD Bass program (identical on all 8 cores)."""
    nc = bacc.Bacc("TRN2", target_bir_lowering=False, debug=False,
                   num_devices=N_CORES)

    xqT = nc.declare_dram_parameter("xqT", [D, L], BF16, isOutput=False)
    xkT = nc.declare_dram_parameter("xkT", [D, L], BF16, isOutput=False)
    xvT = nc.declare_dram_parameter("xvT", [D, L], BF16, isOutput=False)
    wqT = nc.declare_dram_parameter("wqT", [D, CPC], BF16, isOutput=False)
    wkT = nc.declare_dram_parameter("wkT", [D, CPC], BF16, isOutput=False)
    wvT = nc.declare_dram_parameter("wvT", [D, CPC], BF16, isOutput=False)
    woT = nc.declare_dram_parameter("woT", [D, D], BF16, isOutput=False)
    bq_in = nc.declare_dram_parameter("bq", [128, 2], F32, isOutput=False)
    bk_in = nc.declare_dram_parameter("bk", [128, 2], F32, isOutput=False)
    bv_in = nc.declare_dram_parameter("bv", [1, CPC], BF16, isOutput=False)
    bo_in = nc.declare_dram_parameter("bo", [1, D], BF16, isOutput=False)
    masks_in = nc.declare_dram_parameter("masks", [128, mw], BF16, isOutput=False)
    out = nc.declare_dram_parameter("out", [LPC, D], BF16, isOutput=True)

    # per head-pair AllGather bounce buffers (normalized attn)
    ag_in = [nc.dram_tensor(f"ag_in{p}", [128, L], BF16) for p in range(2)]
    ag_out = [nc.dram_tensor(f"ag_out{p}", [4, 128, L], BF16)
              for p in range(2)]
    r_dram = [nc.dram_tensor(f"r_dram{p}", [8, QC], BF16) for p in range(2)]

    with tile.TileContext(nc, num_cores=N_CORES) as tc:
        with tc.tile_pool(name="persist", bufs=1) as pers:
            wq_sb = pers.tile([128, NDB, CPC], BF16, tag="wq")
            wk_sb = pers.tile([128, NDB, CPC], BF16, tag="wk")
            wv_sb = pers.tile([128, NDB, CPC], BF16, tag="wv")
            wo_sb = pers.tile([128, NDB, D], BF16, tag="wo")
            bq_sb = pers.tile([128, 2], F32, tag="bq")
            bk_sb = pers.tile([128, 2], F32, tag="bk")
            bv_sb = pers.tile([1, CPC], BF16, tag="bv")
            bo_sb = pers.tile([1, D], BF16, tag="bo")
            masks_sb = pers.tile([128, mw], BF16, tag="masks")
            ones_sb = pers.tile([1, 128], BF16, tag="ones")
            qT_sb = pers.tile([128, 2, L], BF16, tag="qT")
            kT_sb = pers.tile([128, 2, L], BF16, tag="kT")
            v_sb = pers.tile([128, NKB, HPC, DH + 1], BF16, tag="v")
            ob0_sb = pers.tile([128, 4, 2, 512], BF16, tag="ob0")

            nc.sync.dma_start(
                out=wk_sb[:], in_=wkT.ap().rearrange("(db p) c -> p db c", p=128))
            nc.sync.dma_start(
                out=wq_sb[:], in_=wqT.ap().rearrange("(db p) c -> p db c", p=128))
            nc.gpsimd.dma_start(
                out=wv_sb[:], in_=wvT.ap().rearrange("(db p) c -> p db c", p=128))
            nc.sync.dma_start(out=bq_sb[:], in_=bq_in[:])
            nc.sync.dma_start(out=bk_sb[:], in_=bk_in[:])
            nc.sync.dma_start(out=bv_sb[:], in_=bv_in[:])
            nc.sync.dma_start(out=bo_sb[:], in_=bo_in[:])
            nc.vector.memset(ones_sb[:], 1.0)
            nc.vector.memset(v_sb[:, :, :, DH:DH + 1], 1.0)
            # per-engine register copies of (rank % 4) * 512 for the
            # phase-O AllGather slice loads
            l0r_e = {}
            for _eng in (nc.sync, nc.scalar):
                l0r_e[_eng.engine] = (_eng.partition_id() % 4) * 512

            # masks early on sync (small, needed at attention start); wo
            # is emitted after the x chunk loads so it doesn't delay them
            nc.sync.dma_start(out=masks_sb[:], in_=masks_in[:])

            # x inputs, loaded as one full-L DMA per 128-channel
            # contraction block: 4 KB contiguous lines (fast) and
            # per-db dependency granularity (first matmuls start early)
            with tc.tile_pool(name="xt", bufs=1) as xtp:
                xq_f = xtp.tile([128, NDB, L], BF16, tag="xq")
                xk_f = xtp.tile([128, NDB, L], BF16, tag="xk")
                xv_f = xtp.tile([128, NDB, L], BF16, tag="xv")
                for db in range(NDB):
                    nc.scalar.dma_start(
                        out=xk_f[:, db, :],
                        in_=xkT.ap().rearrange("(db p) l -> p db l",
                                               p=128)[:, db, :])
                for db in range(NDB):
                    nc.sync.dma_start(
                        out=xq_f[:, db, :],
                        in_=xqT.ap().rearrange("(db p) l -> p db l",
                                               p=128)[:, db, :])
                    nc.gpsimd.dma_start(
                        out=xv_f[:, db, :],
                        in_=xvT.ap().rearrange("(db p) l -> p db l",
                                               p=128)[:, db, :])
                wo_r = woT.ap().rearrange("(db p) c -> p db c", p=128)
                nc.scalar.dma_start(out=wo_sb[:, 0:4, :], in_=wo_r[:, 0:4, :])
                nc.gpsimd.dma_start(out=wo_sb[:, 4:8, :], in_=wo_r[:, 4:8, :])

                # ---- emission helpers ----
                def emit_qk_cb(psp, w_sb, b_sb, t_sb, xs, cb, lcs, nm):
                    """One output-channel block (128 ch) of a q/k
                    projection for the given l-chunks; weight stationary."""
                    pss = [psp.tile([128, QC], F32, tag="psqk",
                                    name=f"psqk_{nm}_{cb}_{lc}")
                           for lc in lcs]
                    for db in range(NDB):
                        for i, lc in enumerate(lcs):
                            nc.tensor.matmul(
                                pss[i][:],
                                lhsT=w_sb[:, db, cb * 128:(cb + 1) * 128],
                                rhs=xs[:, db, lc * QC:(lc + 1) * QC],
                                start=(db == 0), stop=(db == NDB - 1))
                    for i, lc in enumerate(lcs):
                        nc.vector.tensor_scalar_add(
                            t_sb[:, cb, lc * QC:(lc + 1) * QC], pss[i][:],
                            b_sb[:, cb:cb + 1])

                def emit_v_ls(psp, lc, ls):
                    """One 128-key block of the V projection."""
                    kbg = lc * 4 + ls
                    psv = psp.tile([128, CPC], F32, tag="psv",
                                   name=f"psv_{kbg}")
                    for db in range(NDB):
                        nc.tensor.matmul(
                            psv[:],
                            lhsT=xv_f[:, db, kbg * 128:(kbg + 1) * 128],
                            rhs=wv_sb[:, db, :],
                            start=(db == 0), stop=False)
                    nc.tensor.matmul(
                        psv[:], lhsT=ones_sb[:, 0:128], rhs=bv_sb[:],
                        start=False, stop=True)
                    nc.vector.tensor_copy(
                        v_sb[:, kbg, :, 0:DH],
                        psv[:].rearrange("p (h d) -> p h d", h=HPC))

                # ---- upfront: K (all), Q(0), V(0) ----
                ctxP = nc.named_scope("phaseP"); ctxP.__enter__()
                with tc.tile_pool(name="psP", bufs=4, space="PSUM") as psP, \
                     tc.tile_pool(name="psPV", bufs=4, space="PSUM") as psPV:
                    # pass 1: K-cb0 and V0 interleaved per contraction
                    # block so the startup PE consumes the xk (scalar)
                    # and xv (gpsimd) DMA streams in parallel
                    pssK = [psP.tile([128, QC], F32, tag="psqk",
                                     name=f"psqk_k0_{lc}")
                            for lc in range(4)]
                    psvs = [psPV.tile([128, CPC], F32, tag="psv",
                                      name=f"psv0_{ls}")
                            for ls in range(4)]
                    for db in range(NDB):
                        for lc in range(4):
                            nc.tensor.matmul(
                                pssK[lc][:],
                                lhsT=wk_sb[:, db, 0:128],
                                rhs=xk_f[:, db, lc * QC:(lc + 1) * QC],
                                start=(db == 0), stop=(db == NDB - 1))
                        for ls in range(4):
                            nc.tensor.matmul(
                                psvs[ls][:],
                                lhsT=xv_f[:, db, ls * 128:(ls + 1) * 128],
                                rhs=wv_sb[:, db, :],
                                start=(db == 0), stop=False)
                    for lc in range(4):
                        nc.vector.tensor_scalar_add(
                            kT_sb[:, 0, lc * QC:(lc + 1) * QC], pssK[lc][:],
                            bk_sb[:, 0:1])
                    for ls in range(4):
                        nc.tensor.matmul(
                            psvs[ls][:], lhsT=ones_sb[:, 0:128],
                            rhs=bv_sb[:], start=False, stop=True)
                        nc.vector.tensor_copy(
                            v_sb[:, ls, :, 0:DH],
                            psvs[ls][:].rearrange("p (h d) -> p h d",
                                                  h=HPC))
                    # pass 2: K-cb1, then Q chunk 0
                    emit_qk_cb(psP, wk_sb, bk_sb, kT_sb, xk_f, 1,
                               [0, 1, 2, 3], "k")
                    for cb in range(2):
                        emit_qk_cb(psP, wq_sb, bq_sb, qT_sb, xq_f, cb, [0],
                                   "q0")
                ctxP.__exit__(None, None, None)

                # ---- attention with woven projection / O-proj work ----
                ctxA = nc.named_scope("phaseA"); ctxA.__enter__()
                with tc.tile_pool(name="ex", bufs=4) as exp_pool, \
                     tc.tile_pool(name="araw", bufs=2) as arawp, \
                     tc.tile_pool(name="sm", bufs=2) as smalls, \
                     tc.tile_pool(name="psS", bufs=2, space="PSUM") as psS, \
                     tc.tile_pool(name="psA", bufs=2, space="PSUM") as psA, \
                     tc.tile_pool(name="psF", bufs=1, space="PSUM") as psF, \
                     tc.tile_pool(name="psF2", bufs=1, space="PSUM") as psF2:

                    fat0 = arawp.tile([128, 4, QC], BF16, tag="fat0")

                    def fill_p0(qc):
                        """Filler chunks during pair-0 attention of chunk
                        qc: project Q/V for chunk qc+1."""
                        if qc >= 3:
                            return []
                        chunks = []
                        for cb in range(2):
                            chunks.append(lambda cb=cb: emit_qk_cb(
                                psF, wq_sb, bq_sb, qT_sb, xq_f, cb,
                                [qc + 1], f"q{qc + 1}"))
                        for ls in range(4):
                            chunks.append(lambda ls=ls: emit_v_ls(
                                psF2, qc + 1, ls))
                        return chunks

                    def o_stage0(ls, nch):
                        """Pair-0 half of the output projection for one
                        output tile; staged to SBUF. Reuses the psqk ring
                        (same shape, disjoint phase) to stay at 8 banks."""
                        po = psF.tile([128, 512], F32, tag="psqk",
                                      name=f"po0_{ls}_{nch}")
                        for r in range(4):
                            nc.tensor.matmul(
                                po[:],
                                lhsT=fat0[:, r, ls * 128:(ls + 1) * 128],
                                rhs=wo_sb[:, 2 * r, nch * 512:(nch + 1) * 512],
                                start=(r == 0), stop=(r == 3))
                        nc.vector.tensor_copy(ob0_sb[:, ls, nch, :], po[:])

                    def fill_p1(qc):
                        if qc == 0:
                            return []
                        # distribute the 8 (ls, nch) tiles: 3 on qc=1,
                        # 3 on qc=2, 2 on qc=3
                        alloc = {1: [(0, 0), (0, 1), (1, 0)],
                                 2: [(1, 1), (2, 0), (2, 1)],
                                 3: [(3, 0), (3, 1)]}[qc]
                        return [lambda ls=ls, nch=nch: o_stage0(ls, nch)
                                for (ls, nch) in alloc]

                    for p in range(2):
                        if p == 1:
                            # own query-slice of pair-0 attn from each rank
                            l0e = l0r_e[nc.sync.engine]
                            for r in range(4):
                                nc.sync.dma_start(
                                    out=fat0[:, r, :],
                                    in_=ag_out[0][r, :, bass.ds(l0e, QC)])
                        for qc in range(NQC):
                            recs = structure[qc]
                            first_kb = recs[0][0]
                            last_kb = recs[-1][0]
                            fillers = fill_p0(qc) if p == 0 else fill_p1(qc)
                            fi = 0
                            n_recs = len(recs)
                            pa = {hp: psA.tile([65, QC], F32, tag="pa",
                                               name=f"pa_{p}_{qc}_{hp}")
                                  for hp in range(2)}
                            for ri, (kb, q0, mask) in enumerate(recs):
                                ps2 = psS.tile([128, 2, QC], F32, tag="ps2",
                                               name=f"ps2_{p}_{qc}_{kb}")
                                for hp in range(2):
                                    h = p * 2 + hp
                                    hb, hoff = h // 2, (h % 2) * 64
                                    nc.tensor.matmul(
                                        ps2[:, hp, q0:],
                                        lhsT=kT_sb[hoff:hoff + 64, hb,
                                                   kb * KB:(kb + 1) * KB],
                                        rhs=qT_sb[hoff:hoff + 64, hb,
                                                  qc * QC + q0:(qc + 1) * QC],
                                        start=True, stop=True)
                                if mask is not None:
                                    off, c0, wm = mask
                                    for hp in range(2):
                                        nc.vector.tensor_add(
                                            ps2[:, hp, c0:c0 + wm],
                                            ps2[:, hp, c0:c0 + wm],
                                            masks_sb[:, off:off + wm])
                                ex2 = exp_pool.tile([128, 2, QC], BF16,
                                                    tag="ex",
                                                    name=f"ex_{p}_{qc}_{kb}")
                                nc.scalar.activation(
                                    out=ex2[:, :, q0:], in_=ps2[:, :, q0:],
                                    func=ExpFn, scale=0.125)
                                # independent PE filler lands in the
                                # exp-wait gap before this rec's AV
                                while (fi < len(fillers)
                                       and fi <= (ri * len(fillers))
                                       // n_recs):
                                    fillers[fi](); fi += 1
                                for hp in range(2):
                                    h = p * 2 + hp
                                    nc.tensor.matmul(
                                        pa[hp][:, q0:],
                                        lhsT=v_sb[:, kb, h, :],
                                        rhs=ex2[:, hp, q0:],
                                        start=(kb == first_kb),
                                        stop=(kb == last_kb))
                            while fi < len(fillers):
                                fillers[fi](); fi += 1
                            # incremental normalize + ship for this qc so
                            # the AllGather can trigger right after the
                            # last attention matmul of the pair
                            araw2 = arawp.tile([64, 2, QC], BF16,
                                               tag="araw",
                                               name=f"araw_{p}_{qc}")
                            s2 = smalls.tile([2, QC], F32, tag="s2",
                                             name=f"s2_{p}_{qc}")
                            for hp in range(2):
                                nc.vector.tensor_copy(
                                    araw2[:, hp, :], pa[hp][0:64, :])
                                stmp = smalls.tile(
                                    [65, QC], F32, tag="stmp",
                                    name=f"stmp_{p}_{qc}_{hp}")
                                nc.vector.tensor_copy(
                                    stmp[64:65, :], pa[hp][64:65, :])
                                nc.gpsimd.dma_start(
                                    out=s2[hp:hp + 1, :],
                                    in_=stmp[64:65, :])
                            r2 = smalls.tile([2, QC], BF16, tag="r2",
                                             name=f"r2_{p}_{qc}")
                            with nc.allow_low_precision(
                                    reason="1/S to bf16; 0.4% on softmax "
                                           "norm is within tolerance"):
                                nc.vector.reciprocal(r2[:], s2[:])
                            nc.sync.dma_start(
                                out=r_dram[p][qc * 2:(qc + 1) * 2, :],
                                in_=r2[:])
                            bc2 = arawp.tile([64, 2, QC], BF16, tag="bc",
                                             name=f"bc_{p}_{qc}")
                            nc.sync.dma_start(
                                out=bc2[:],
                                in_=bass.AP(tensor=r_dram[p],
                                            offset=qc * 2 * QC,
                                            ap=[[0, 64], [QC, 2], [1, QC]]))
                            fn2 = arawp.tile([64, 2, QC], BF16, tag="fn",
                                             name=f"fn_{p}_{qc}")
                            nc.vector.tensor_mul(
                                fn2[:].rearrange("p a l -> p (a l)"),
                                araw2[:].rearrange("p a l -> p (a l)"),
                                bc2[:].rearrange("p a l -> p (a l)"))
                            for hp in range(2):
                                nc.gpsimd.dma_start(
                                    out=ag_in[p][hp * 64:(hp + 1) * 64,
                                                 qc * QC:(qc + 1) * QC],
                                    in_=fn2[:, hp, :])
                        nc.gpsimd.collective_compute(
                            "AllGather", mybir.AluOpType.bypass,
                            replica_groups=GROUPS,
                            ins=[ag_in[p][:]], outs=[ag_out[p][:]])

                ctxA.__exit__(None, None, None)
            # ---------------- Phase O: finish output projection ---------
            ctxO = nc.named_scope("phaseO"); ctxO.__enter__()
            with tc.tile_pool(name="fat", bufs=1) as fatp, \
                 tc.tile_pool(name="ob", bufs=4) as obp, \
                 tc.tile_pool(name="psO", bufs=4, space="PSUM") as psO:
                fat1 = fatp.tile([128, 4, QC], BF16, tag="fat1")
                l0e = l0r_e[nc.scalar.engine]
                for r in range(4):
                    nc.scalar.dma_start(
                        out=fat1[:, r, :],
                        in_=ag_out[1][r, :, bass.ds(l0e, QC)])
                for ls in range(4):
                    for nch in range(2):
                        po = psO.tile([128, 512], F32, tag="po1",
                                      name=f"po1_{ls}_{nch}")
                        for r in range(4):
                            nc.tensor.matmul(
                                po[:],
                                lhsT=fat1[:, r, ls * 128:(ls + 1) * 128],
                                rhs=wo_sb[:, 2 * r + 1,
                                          nch * 512:(nch + 1) * 512],
                                start=(r == 0), stop=False)
                        nc.tensor.matmul(
                            po[:], lhsT=ones_sb[:, 0:128],
                            rhs=bo_sb[:, nch * 512:(nch + 1) * 512],
                            start=False, stop=True)
                        ob = obp.tile([128, 512], BF16, tag="ob",
                                      name=f"ob_{ls}_{nch}")
                        nc.vector.tensor_add(
                            ob[:], po[:], ob0_sb[:, ls, nch, :])
                        oeng = nc.sync if nch == 0 else nc.scalar
                        oeng.dma_start(
                            out=out[ls * 128:(ls + 1) * 128,
                                    nch * 512:(nch + 1) * 512],
                            in_=ob[:])

    ctxO.__exit__(None, None, None)
    nc.compile()
    return nc


def _host_fallback(query, key, value, attn_mask, key_padding_mask,
                   Wq, bq, Wk, bk, Wv, bv, Wo, bo):
    """Exact fp32 numpy replica of the reference (degenerate masks only)."""
    q = (query @ Wq.T + bq).reshape(B, L, H, DH).transpose(0, 2, 1, 3)
    k = (key @ Wk.T + bk).reshape(B, L, H, DH).transpose(0, 2, 1, 3)
    v = (value @ Wv.T + bv).reshape(B, L, H, DH).transpose(0, 2, 1, 3)
    scores = np.einsum('bhqd,bhkd->bhqk', q, k) / np.sqrt(np.float32(DH))
    scores = np.where(key_padding_mask[:, None, None, :], -1e30, scores)
    scores = np.where(attn_mask[None, None, :, :], -1e30, scores)
    scores = scores - scores.max(axis=-1, keepdims=True)
    w = np.exp(scores)
    w = w / w.sum(axis=-1, keepdims=True)
    attn = np.einsum('bhqk,bhkd->bhqd', w, v)
    attn = attn.transpose(0, 2, 1, 3).reshape(B, L, D)
    return (attn @ Wo.T + bo).astype(np.float32)


def kernel(query, key, value, attn_mask, key_padding_mask,
           Wq, bq, Wk, bk, Wv, bv, Wo, bo):
    global last_results
    query = np.asarray(query, dtype=np.float32)
    key = np.asarray(key, dtype=np.float32)
    value = np.asarray(value, dtype=np.float32)
    attn_mask = np.asarray(attn_mask, dtype=bool)
    key_padding_mask = np.asarray(key_padding_mask, dtype=bool)
    Wq, bq = np.asarray(Wq, np.float32), np.asarray(bq, np.float32)
    Wk, bk = np.asarray(Wk, np.float32), np.asarray(bk, np.float32)
    Wv, bv = np.asarray(Wv, np.float32), np.asarray(bv, np.float32)
    Wo, bo = np.asarray(Wo, np.float32), np.asarray(bo, np.float32)

    structure, mask_bufs, degenerate = _analyze_masks(attn_mask,
                                                      key_padding_mask)
    if degenerate:
        return _host_fallback(query, key, value, attn_mask, key_padding_mask,
                              Wq, bq, Wk, bk, Wv, bv, Wo, bo)

    mw = mask_bufs[0].shape[1]
    key_sig = _structure_key(structure, mw)
    if key_sig not in _PROG_CACHE:
        _PROG_CACHE[key_sig] = _build_program(structure, mw)
    nc = _PROG_CACHE[key_sig]

    woT_np = np.ascontiguousarray(Wo.T).astype(NPBF16)
    bo_np = bo.reshape(1, D).astype(NPBF16)
    xT_bf = [np.ascontiguousarray(a.transpose(0, 2, 1)).astype(NPBF16)
             for a in (query, key, value)]             # [B, D, L] bf16

    in_maps = []
    for core in range(N_CORES):
        b, j = divmod(core, 4)
        csl = slice(j * CPC, (j + 1) * CPC)
        in_maps.append({
            "xqT": xT_bf[0][b],
            "xkT": xT_bf[1][b],
            "xvT": xT_bf[2][b],
            "wqT": np.ascontiguousarray(Wq[csl, :].T).astype(NPBF16),
            "wkT": np.ascontiguousarray(Wk[csl, :].T).astype(NPBF16),
            "wvT": np.ascontiguousarray(Wv[csl, :].T).astype(NPBF16),
            "woT": woT_np,
            "bq": np.ascontiguousarray(bq[csl].reshape(2, 128).T),
            "bk": np.ascontiguousarray(bk[csl].reshape(2, 128).T),
            "bv": bv[csl].reshape(1, CPC).astype(NPBF16),
            "bo": bo_np,
            "masks": mask_bufs[b].astype(NPBF16),
        })

    trace = os.environ.get("KERNEL_TRACE", "0") == "1"
    res = run_bass_kernel_spmd(nc, in_maps, list(range(N_CORES)), trace=trace)
    last_results = res

    out = np.empty((B, L, D), dtype=np.float32)
    for core in range(N_CORES):
        b, j = divmod(core, 4)
        out[b, j * LPC:(j + 1) * LPC, :] = res.results[core]["out"].astype(np.float32)
    return out
